# revision 20
# baseline (speedup 1.0000x reference)
"""DETR-style matcher cost matrix on 8 Trainium2 NeuronCores — v3.

cost[b, g, p] = -pred_cls[b, p, g]
                + mean(|pred_box[p] - gt_box[g]|)          (L1, 4 coords)
                + 1 - IoU + (area_c - union)/(area_c+eps)  (GIoU loss)
masked to zero where gt_validity[b, g] == 0.

Sharding: data-parallel over batch, 4 batches per core (B=32, 8 cores).
Layout per unit: [128 part = gt rows, 900 free = preds], fp16 throughout.

v5: all elementwise work is custom DVE ops authored WITH 2x perf-mode uop
programs (perf_max in byte-36 + 2x variants in the per-NEFF DVE table), so
each [128,900] op runs at ~630ns instead of ~1150ns; the no-scalar ops and
the Act reciprocals are additionally fused across unit PAIRS as [128,1800]
instructions to amortize per-instruction overhead.  Host maps are
negate-scaled (Pm = -0.5*P, Gm = -0.5*G) so every PE pass uses the same +1
identity stationary:
  wi02n = max(Pm2x2,Gm2x2) - min(Pm2x1,Gm2x1)  [W0CF]  (= -0.25*2*wi0)
  wc2n  = min(Pm2x2,Gm2x2) - max(Pm2x1,Gm2x1)  [W0F]   (= -0.25*2*wc)
  inter16 = min(wi02n,0)*min(hi02n,0)          [MINMUL] (= inter/4)
  union4e = (4Ap + (4Ag+4eps)) - 16*inter16    [SUBSCM]
  areac4  = 16*wc2n*hc2n                       [MULIMM imm=16]
  rcu4  = 1/union4e, rca4N = -1/(areac4+4eps)  [Act Reciprocal]
  iouN  = -16*inter16*rcu4 = -iou              [MULIMM imm=-16]
  t2mN  = union4e*rca4N = -(union+eps)/(areac+eps)  [MULIMM]
PE (5 passes, all stationary +1*I): psum = clsm + wi02n + hi02n + iouN +
  t2mN, where clsm = -cls^T + 0.25*(Wp+Hp) (host).  l1's per-gt part + the
  GIoU "+2" ride the out-op bias: out = V*psum + V*(2 + 0.25*(Wg+Hg)).
"""

import numpy as np

B, Q = 32, 900
N_CORES = 8
B_PER = B // N_CORES
EPS = 1e-7
_cached = {}


def _split_multi_waits(nc):
    """This neuronxcc build rejects >1 sync-wait per instruction. Split any
    instruction carrying N>1 waits by inserting N-1 wait-carrier nops before
    it on the same (in-order) engine stream."""
    import concourse.mybir as mybir

    for fn in nc.m.functions:
        for bb in fn.blocks:
            out = []
            for ins in bb.instructions:
                si = getattr(ins, "sync_info", None)
                waits = list(si.on_wait) if (si and si.on_wait) else []
                if len(waits) > 1:
                    si.on_wait = [waits[-1]]
                    for j, w in enumerate(waits[:-1]):
                        nop = mybir.InstNoOp(name=f"{ins.name}-sw{j}", ins=[], outs=[])
                        nop.engine = ins.engine
                        nop.sync_info = mybir.SyncInfo(on_wait=[w], on_update=[])
                        out.append(nop)
                out.append(ins)
            bb.instructions[:] = out


def _ensure_custom_ops():
    """Author the fused DVE ops with hand-written 2X perf-mode uop programs
    (stock-table idiom: results parked in delay lanes, WR0_LO/HI select them)
    and register them in dve_ops' tables + compile cache."""
    from concourse import dve_ops
    from concourse.dve_spec import (
        Spec, Src0, Src1, C0, C1, C2, Zero, minn, maxx, relu, lower, _has_src1,
    )
    from concourse.dve_uop import (
        DveOpSpec, UopConfig, UopDpConfig, InpSel, OutPath, OutSel, AluOp,
        AluInp, DelayInp,
    )

    A = AluInp
    D = {i: getattr(A, f"PREV_DELAY_{i}") for i in range(6)}
    PA = A.PREV_ALU_OUT

    def mk2x(lanes, blocks, lo_out, hi_out):
        u = UopConfig()
        for i, s in enumerate(lanes):
            if s is not None:
                u.enable_input(s, i)
        dps = []
        for (op, s0, s1, dmap) in blocks:
            dp = UopDpConfig()
            if op is not None:
                dp.op = op
                dp.alu_src0 = s0
                dp.alu_src1 = s1
                dp.alu_out_enable = 1
            else:
                dp.alu_out_enable = 0
            dp.delay = [DelayInp.PREV_DELAY] * 7
            dp.delay_enable = [0] * 7
            for lane, sel in dmap.items():
                dp.delay[lane] = DelayInp(sel)
                dp.delay_enable[lane] = 1
            dps.append(dp)
        assert len(dps) == 8
        u.datapath_config = dps
        u.enable_output(lo_out, OutPath.WR0_LO)
        u.enable_output(hi_out, OutPath.WR0_HI)
        return u

    # min/max-pair ops: lanes 0=x 1->d0=y 2->d1=c0 3->d2=c1 4->d3=xh 5->d4=yh
    def u2x_minmax(op_a, op_b):
        return mk2x(
            [InpSel.SRC_0, InpSel.SRC_1, InpSel.CONST_0, InpSel.CONST_1,
             InpSel.SRC_0_HI, InpSel.SRC_1_HI, None],
            [
                (op_a, PA, D[1], {0: 5, 1: 5, 2: 5, 3: 5, 4: 5}),
                (op_b, D[0], D[2], {1: 5, 2: 5, 3: 5, 4: 5, 5: 0}),
                (AluOp.SUBTRACT, D[5], PA, {1: 5, 2: 5, 3: 5, 4: 5}),
                (op_a, D[3], D[1], {2: 5, 4: 5, 5: 0}),
                (op_b, D[4], D[2], {5: 5, 0: 0}),
                (AluOp.SUBTRACT, D[0], PA, {5: 5}),
                (None, None, None, {5: 5, 0: 0}),
                (None, None, None, {5: 5, 0: 5}),
            ],
            OutSel.DELAY_5, OutSel.DELAY_0,
        )

    # MULIMM: x*y*c2 ; lanes 0=x 1->d0=y 2->d1=c2 3->d2=xh 4->d3=yh
    u2x_mul = mk2x(
        [InpSel.SRC_0, InpSel.SRC_1, InpSel.CONST_2, InpSel.SRC_0_HI,
         InpSel.SRC_1_HI, None, None],
        [
            (AluOp.MULTIPLY, PA, D[0], {1: 5, 2: 5, 3: 5}),
            (AluOp.MULTIPLY, PA, D[1], {1: 5, 2: 5, 3: 5}),
            (AluOp.MULTIPLY, D[2], D[3], {1: 5, 4: 0}),
            (AluOp.MULTIPLY, PA, D[1], {4: 5}),
            (None, None, None, {4: 5, 5: 0}),
            (None, None, None, {4: 5, 5: 5}),
            (None, None, None, {4: 5, 5: 5}),
            (None, None, None, {4: 5, 5: 5}),
        ],
        OutSel.DELAY_4, OutSel.DELAY_5,
    )

    # RELUMULF: relu(x)*relu(y) ; lanes 0=x 1->d0=y 2->d1=ZERO 3->d2=xh 4->d3=yh
    u2x_rm = mk2x(
        [InpSel.SRC_0, InpSel.SRC_1, InpSel.ZERO, InpSel.SRC_0_HI,
         InpSel.SRC_1_HI, None, None],
        [
            (AluOp.MAX, PA, D[1], {0: 5, 1: 5, 2: 5, 3: 5}),
            (AluOp.MAX, D[0], D[1], {1: 5, 2: 5, 3: 5, 5: 0}),
            (AluOp.MULTIPLY, D[5], PA, {1: 5, 2: 5, 3: 5}),
            (AluOp.MAX, D[2], D[1], {1: 5, 3: 5, 4: 0}),
            (AluOp.MAX, D[3], D[1], {4: 5, 0: 0}),
            (AluOp.MULTIPLY, D[0], PA, {4: 5}),
            (None, None, None, {4: 5, 5: 0}),
            (None, None, None, {4: 5, 5: 5}),
        ],
        OutSel.DELAY_4, OutSel.DELAY_5,
    )

    # SUBSC: (x + c0) - y ; lanes 0=x 1->d0=y 2->d1=c0 3->d2=xh 4->d3=yh
    u2x_sub = mk2x(
        [InpSel.SRC_0, InpSel.SRC_1, InpSel.CONST_0, InpSel.SRC_0_HI,
         InpSel.SRC_1_HI, None, None],
        [
            (AluOp.ADD, PA, D[1], {0: 5, 1: 5, 2: 5, 3: 5}),
            (AluOp.SUBTRACT, PA, D[0], {1: 5, 2: 5, 3: 5}),
            (AluOp.ADD, D[2], D[1], {3: 5, 4: 0}),
            (AluOp.SUBTRACT, PA, D[3], {4: 5}),
            (None, None, None, {4: 5, 5: 0}),
            (None, None, None, {4: 5, 5: 5}),
            (None, None, None, {4: 5, 5: 5}),
            (None, None, None, {4: 5, 5: 5}),
        ],
        OutSel.DELAY_4, OutSel.DELAY_5,
    )

    # MINMUL: min(x,0)*min(y,0) ; lanes 0=x 1->d0=y 2->d1=ZERO 3->d2=xh 4->d3=yh
    u2x_mm = mk2x(
        [InpSel.SRC_0, InpSel.SRC_1, InpSel.ZERO, InpSel.SRC_0_HI,
         InpSel.SRC_1_HI, None, None],
        [
            (AluOp.MIN, PA, D[1], {0: 5, 1: 5, 2: 5, 3: 5}),
            (AluOp.MIN, D[0], D[1], {1: 5, 2: 5, 3: 5, 5: 0}),
            (AluOp.MULTIPLY, D[5], PA, {1: 5, 2: 5, 3: 5}),
            (AluOp.MIN, D[2], D[1], {1: 5, 3: 5, 4: 0}),
            (AluOp.MIN, D[3], D[1], {4: 5, 0: 0}),
            (AluOp.MULTIPLY, D[0], PA, {4: 5}),
            (None, None, None, {4: 5, 5: 0}),
            (None, None, None, {4: 5, 5: 5}),
        ],
        OutSel.DELAY_4, OutSel.DELAY_5,
    )

    # SUBSCM: (x + c0) - y*c2
    # lanes: 0=y 1->d0=x 2->d1=c0 3->d2=c2 4->d3=yh 5->d4=xh
    u2x_subm = mk2x(
        [InpSel.SRC_1, InpSel.SRC_0, InpSel.CONST_0, InpSel.CONST_2,
         InpSel.SRC_1_HI, InpSel.SRC_0_HI, None],
        [
            (AluOp.MULTIPLY, PA, D[2], {0: 5, 1: 5, 2: 5, 3: 5, 4: 5}),
            (AluOp.ADD, D[0], D[1], {1: 5, 2: 5, 3: 5, 4: 5, 5: 0}),
            (AluOp.SUBTRACT, PA, D[5], {1: 5, 2: 5, 3: 5, 4: 5}),
            (AluOp.MULTIPLY, D[3], D[2], {1: 5, 4: 5, 5: 0}),
            (AluOp.ADD, D[4], D[1], {5: 5, 0: 0}),
            (AluOp.SUBTRACT, PA, D[0], {5: 5}),
            (None, None, None, {5: 5, 0: 0}),
            (None, None, None, {5: 5, 0: 5}),
        ],
        OutSel.DELAY_5, OutSel.DELAY_0,
    )

    # ADDF: x + y ; lanes 0=x 1->d0=y 2->d1=xh 3->d2=yh
    u2x_addf = mk2x(
        [InpSel.SRC_0, InpSel.SRC_1, InpSel.SRC_0_HI, InpSel.SRC_1_HI,
         None, None, None],
        [
            (AluOp.ADD, PA, D[0], {0: 5, 1: 5, 2: 5}),
            (AluOp.ADD, D[1], D[2], {3: 0}),
            (None, None, None, {3: 5, 4: 0}),
            (None, None, None, {3: 5, 4: 5}),
            (None, None, None, {3: 5, 4: 5}),
            (None, None, None, {3: 5, 4: 5}),
            (None, None, None, {3: 5, 4: 5}),
            (None, None, None, {3: 5, 4: 5}),
        ],
        OutSel.DELAY_3, OutSel.DELAY_4,
    )

    def author(name, body, ref, u2):
        if name in dve_ops._SUB_OPCODE_FOR_NAME:
            return
        spec = Spec(body=body, reference=ref)
        row = max(dve_ops._SUB_OPCODE_FOR_NAME.values()) + 1
        shas = {}
        for ver in ("v3", "v4"):
            uops = lower(spec, ver=ver)
            assert len(uops) == 1, (name, len(uops))
            import copy as _copy

            u2v = _copy.deepcopy(u2)
            u2v.trigger = uops[0].trigger
            u2v.next_uop = uops[0].next_uop
            u2v.repeat_count = uops[0].repeat_count
            u2v.require_inp0 = uops[0].require_inp0
            u2v.require_inp1 = uops[0].require_inp1
            s = DveOpSpec(
                name=name, opcode=row, uops=uops,
                uops_2x=[u2v], uops_2x_2p=[u2v], uops_4x=[u2v],
                perf_max=3, rd1_en=_has_src1(spec),
            )
            shas[ver] = s.sha(ver)
            dve_ops._COMPILE_CACHE[(name, ver)] = s
        op = dve_ops.DveOp(name, spec, False, shas)
        dve_ops.OPS.append(op)
        dve_ops.CUSTOM_DVE_SPECS[name] = spec
        dve_ops._SUB_OPCODE_FOR_NAME[name] = row

    author(
        "W0F_ANT",
        minn(Src0, C0) - maxx(Src1, C1),
        lambda in0, in1, s0, s1, imm2: np.minimum(in0, s0) - np.maximum(in1, s1),
        u2x_minmax(AluOp.MIN, AluOp.MAX),
    )
    author(
        "W0CF_ANT",
        maxx(Src0, C0) - minn(Src1, C1),
        lambda in0, in1, s0, s1, imm2: np.maximum(in0, s0) - np.minimum(in1, s1),
        u2x_minmax(AluOp.MAX, AluOp.MIN),
    )
    author(
        "RELUMULF_ANT",
        relu(Src0) * relu(Src1),
        lambda in0, in1, s0, s1, imm2: np.maximum(in0, 0.0) * np.maximum(in1, 0.0),
        u2x_rm,
    )
    author(
        "SUBSC_ANT",
        (Src0 + C0) - Src1,
        lambda in0, in1, s0, s1, imm2: (in0 + s0) - in1,
        u2x_sub,
    )
    author(
        "MINMUL_ANT",
        minn(Src0, Zero) * minn(Src1, Zero),
        lambda in0, in1, s0, s1, imm2: np.minimum(in0, 0.0) * np.minimum(in1, 0.0),
        u2x_mm,
    )
    author(
        "SUBSCM_ANT",
        (Src0 + C0) - Src1 * C2,
        lambda in0, in1, s0, s1, imm2: (in0 + s0) - in1 * imm2,
        u2x_subm,
    )
    author(
        "ADDF_ANT",
        Src0 + Src1,
        lambda in0, in1, s0, s1, imm2: in0 + in1,
        u2x_addf,
    )
    author(
        "MULIMM_ANT",
        Src0 * Src1 * C2,
        lambda in0, in1, s0, s1, imm2: in0 * in1 * imm2,
        u2x_mul,
    )


def _by_name(dve_ops, name):
    for op in dve_ops.OPS:
        if op.name == name:
            return op
    raise KeyError(name)


def _raw_act(nc, out, in_, func, bias=0.0, scale=1.0):
    """activation() without the Reciprocal accuracy ban (measured ~1e-5)."""
    import concourse.bass as bass
    from concourse import mybir

    inputs = [nc.scalar.lower_ap(in_)]
    for arg in (bias, scale, 0.0):
        if isinstance(arg, bass.AP):
            inputs.append(nc.scalar.lower_ap(arg))
        else:
            inputs.append(mybir.ImmediateValue(dtype=mybir.dt.float32, value=arg))
    return nc.scalar.add_instruction(
        mybir.InstActivation(
            name=nc.get_next_instruction_name(),
            func=func,
            ins=inputs,
            outs=[nc.scalar.lower_ap(out)],
        )
    )


def _build_nc():
    import concourse.bass as bass
    from concourse import mybir, dve_ops, bass_isa
    from concourse.tile import TileContext
    from concourse.masks import make_identity

    _ensure_custom_ops()
    W0F = _by_name(dve_ops, "W0F_ANT")
    W0CF = _by_name(dve_ops, "W0CF_ANT")
    RM = _by_name(dve_ops, "MINMUL_ANT")
    SUB = _by_name(dve_ops, "SUBSCM_ANT")
    MUL = _by_name(dve_ops, "MULIMM_ANT")
    ADDF = _by_name(dve_ops, "ADDF_ANT")

    f32 = mybir.dt.float32
    f16 = mybir.dt.float16
    Act = mybir.ActivationFunctionType

    nc = bass.Bass()
    # host-packed inputs
    cls_d = nc.dram_tensor("clsm", [B_PER, Q, Q], f16, kind="ExternalInput")
    pmap_d = nc.dram_tensor("pmap", [B_PER, 5, Q], f16, kind="ExternalInput")
    gsc_d = nc.dram_tensor("gsc", [B_PER, Q, 8], f32, kind="ExternalInput")
    cost_d = nc.dram_tensor("cost", [B_PER, Q, Q], f16, kind="ExternalOutput")

    with TileContext(nc) as tc:
        with (
            tc.tile_pool(name="const", bufs=1) as constp,
            tc.tile_pool(name="batch", bufs=3) as batchp,
            tc.tile_pool(name="unit", bufs=3) as up,
            tc.tile_pool(name="cls", bufs=6) as clsp,
            tc.tile_pool(name="outp", bufs=4) as outp,
            tc.tile_pool(name="ps", bufs=4, space="PSUM") as psp,
        ):
            # stationary matrices for PE accumulate passes
            ident = constp.tile([128, 128], f32)
            make_identity(nc, ident)
            stat_p1 = constp.tile([128, 128], f16)
            nc.vector.tensor_scalar_mul(stat_p1[:], ident[:], 1.0)

            def emit_scalar_stage(mp, gsc, wi02, hi02, wc2, hc2):
                """Per-unit scalar-carrying customs into (slices of) tiles."""
                G2x1 = gsc[:, 0:1]
                G2y1 = gsc[:, 1:2]
                G2x2 = gsc[:, 2:3]
                G2y2 = gsc[:, 3:4]
                cd = nc.vector._custom_dve
                cd(W0CF, out=wi02, in0=mp["P2x2"], in1=mp["P2x1"],
                   s0=G2x2, s1=G2x1)
                cd(W0CF, out=hi02, in0=mp["P2y2"], in1=mp["P2y1"],
                   s0=G2y2, s1=G2y1)
                cd(W0F, out=wc2, in0=mp["P2x2"], in1=mp["P2x1"],
                   s0=G2x2, s1=G2x1)
                cd(W0F, out=hc2, in0=mp["P2y2"], in1=mp["P2y1"],
                   s0=G2y2, s1=G2y1)

            def emit_pe_out(cls, gsc, out, wi02, hi02, iouN, t2mN):
                """PE accumulation + V-scale out for one unit; inputs are
                [128,900] APs (possibly slices of fused double tiles).
                cls is pre-added into t2mC on the DVE (4 PE passes)."""
                V = gsc[:, 5:6]
                bV = gsc[:, 6:7]
                t2mC = up.tile([128, Q], f16, tag="t2mC")
                nc.vector._custom_dve(ADDF, out=t2mC[:], in0=t2mN, in1=cls)
                psA = psp.tile([128, 512], f32, tag="psA")
                psB = psp.tile([128, Q - 512], f32, tag="psB")
                passes = [
                    (stat_p1, wi02, True),
                    (stat_p1, hi02, False),
                    (stat_p1, iouN, False),
                    (stat_p1, t2mC[:], False),
                ]
                for k, (st, x, first) in enumerate(passes):
                    last = k == len(passes) - 1
                    nc.tensor.matmul(
                        psA[:], st[:], x[:, 0:512], start=first, stop=last,
                        skip_group_check=True,
                    )
                    nc.tensor.matmul(
                        psB[:], st[:], x[:, 512:Q], start=first, stop=last,
                        skip_group_check=True,
                    )
                nc.scalar.activation(
                    out[:, 0:512], psA[:], Act.Identity, scale=V, bias=bV
                )
                nc.scalar.activation(
                    out[:, 512:Q], psB[:], Act.Identity, scale=V, bias=bV
                )

            def emit_pair(ua, ub):
                """Two units fused: the no-scalar customs + Act reciprocals
                run once over [128,1800] double-width tiles."""
                cd = nc.vector._custom_dve
                Q2 = 2 * Q
                wi02 = up.tile([128, Q2], f16, tag="wi02")
                hi02 = up.tile([128, Q2], f16, tag="hi02")
                wc2 = up.tile([128, Q2], f16, tag="wc2")
                hc2 = up.tile([128, Q2], f16, tag="hc2")
                for (mp, cls, gsc, out, store), o in ((ua, 0), (ub, Q)):
                    emit_scalar_stage(
                        mp, gsc, wi02[:, o : o + Q], hi02[:, o : o + Q],
                        wc2[:, o : o + Q], hc2[:, o : o + Q],
                    )
                inter4 = up.tile([128, Q2], f16, tag="inter4")
                cd(RM, out=inter4[:], in0=wi02[:], in1=hi02[:])
                areac4 = up.tile([128, Q2], f16, tag="areac4")
                cd(MUL, out=areac4[:], in0=wc2[:], in1=hc2[:], imm2=16.0)
                union4e = up.tile([128, Q2], f16, tag="union4e")
                for (mp, cls, gsc, out, store), o in ((ua, 0), (ub, Q)):
                    cd(SUB, out=union4e[:, o : o + Q], in0=mp["Ap4"],
                       in1=inter4[:, o : o + Q], s0=gsc[:, 4:5], imm2=16.0)
                rcu4 = up.tile([128, Q2], f16, tag="rcu4")
                _raw_act(nc, rcu4[:], union4e[:], Act.Reciprocal)
                rca4N = up.tile([128, Q2], f16, tag="rca4N")
                _raw_act(nc, rca4N[:], areac4[:], Act.Reciprocal,
                         bias=-4 * EPS, scale=-1.0)
                iouN = up.tile([128, Q2], f16, tag="iouN")
                cd(MUL, out=iouN[:], in0=inter4[:], in1=rcu4[:], imm2=-16.0)
                t2mN = up.tile([128, Q2], f16, tag="t2mN")
                cd(MUL, out=t2mN[:], in0=union4e[:], in1=rca4N[:], imm2=1.0)
                for (mp, cls, gsc, out, store), o in ((ua, 0), (ub, Q)):
                    emit_pe_out(
                        cls, gsc, out, wi02[:, o : o + Q], hi02[:, o : o + Q],
                        iouN[:, o : o + Q], t2mN[:, o : o + Q],
                    )
                    store()

            def emit_unit(mp, cls, gsc, out, tag):
                """Unfused single unit (packed remainder)."""
                cd = nc.vector._custom_dve
                wi02 = constp.tile([128, Q], f16, tag="swi02")
                hi02 = constp.tile([128, Q], f16, tag="shi02")
                wc2 = constp.tile([128, Q], f16, tag="swc2")
                hc2 = constp.tile([128, Q], f16, tag="shc2")
                emit_scalar_stage(mp, gsc, wi02[:], hi02[:], wc2[:], hc2[:])
                inter4 = constp.tile([128, Q], f16, tag="sinter4")
                cd(RM, out=inter4[:], in0=wi02[:], in1=hi02[:])
                union4e = constp.tile([128, Q], f16, tag="sunion4e")
                cd(SUB, out=union4e[:], in0=mp["Ap4"], in1=inter4[:],
                   s0=gsc[:, 4:5], imm2=16.0)
                areac4 = constp.tile([128, Q], f16, tag="sareac4")
                cd(MUL, out=areac4[:], in0=wc2[:], in1=hc2[:], imm2=16.0)
                rcu4 = constp.tile([128, Q], f16, tag="srcu4")
                _raw_act(nc, rcu4[:], union4e[:], Act.Reciprocal)
                rca4N = constp.tile([128, Q], f16, tag="srca4N")
                _raw_act(nc, rca4N[:], areac4[:], Act.Reciprocal,
                         bias=-4 * EPS, scale=-1.0)
                iouN = constp.tile([128, Q], f16, tag="siouN")
                cd(MUL, out=iouN[:], in0=inter4[:], in1=rcu4[:], imm2=-16.0)
                t2mN = constp.tile([128, Q], f16, tag="st2mN")
                cd(MUL, out=t2mN[:], in0=union4e[:], in1=rca4N[:], imm2=1.0)
                emit_pe_out(cls, gsc, out, wi02[:], hi02[:], iouN[:], t2mN[:])

            # ---------------- packed remainder unit ----------------
            # partitions 4b..4b+3 = batch b rows 896:900; rest = filler
            def emit_packed():
                mpR = {}
                for k, nm in enumerate(("P2x1", "P2y1", "P2x2", "P2y2", "Ap4")):
                    t = constp.tile([128, Q], f16, tag=f"mapR{nm}", name=f"mapR{nm}")
                    for b in range(B_PER):
                        src = pmap_d[b, k][:]
                        rows = 4 if b < 3 else 116
                        bcast = bass.AP(
                            tensor=src.tensor, offset=src.offset,
                            ap=[[0, rows]] + list(src.ap),
                        )
                        nc.sync.dma_start(out=t[4 * b : 4 * b + rows, :], in_=bcast)
                    mpR[nm] = t[:]

                gscR = constp.tile([128, 8], f32, tag="gscR")
                # filler gt box: G2x1=0.2, G2x2=0.3 etc keeps union/areac
                # positive so the Reciprocals stay finite on filler rows
                nc.gpsimd.memset(gscR[:], 0.2)
                nc.gpsimd.memset(gscR[:, 2:4], 0.3)
                for b in range(B_PER):
                    nc.sync.dma_start(
                        out=gscR[4 * b : 4 * b + 4, :], in_=gsc_d[b, 896:900, :]
                    )
                clsR = constp.tile([128, Q], f16, tag="clsR")
                nc.gpsimd.memset(clsR[:], 0.0)
                for b in range(B_PER):
                    nc.sync.dma_start(
                        out=clsR[4 * b : 4 * b + 4, :], in_=cls_d[b, 896:900, :]
                    )
                outR = outp.tile([128, Q], f16, tag="outR")
                emit_unit(mpR, clsR[:], gscR[:], outR, "packed")
                for b in range(B_PER):
                    nc.scalar.dma_start(
                        out=cost_d[b, 896:900, :], in_=outR[4 * b : 4 * b + 4, :]
                    )

            # ---------------- full units, emitted in fused pairs --------
            pend = None
            for b in range(B_PER):
                mp = {}
                for k, nm in enumerate(("P2x1", "P2y1", "P2x2", "P2y2", "Ap4")):
                    t = batchp.tile([128, Q], f16, tag=f"map{nm}", name=f"map{nm}")
                    src = pmap_d[b, k][:]
                    bcast = bass.AP(
                        tensor=src.tensor, offset=src.offset,
                        ap=[[0, 128]] + list(src.ap),
                    )
                    nc.sync.dma_start(out=t[:], in_=bcast)
                    mp[nm] = t[:]

                for t in range(7):
                    g0 = t * 128
                    cls = clsp.tile([128, Q], f16, tag="cls")
                    nc.sync.dma_start(out=cls[:], in_=cls_d[b, g0 : g0 + 128, :])
                    gsc = clsp.tile([128, 8], f32, tag="gsc")
                    nc.sync.dma_start(out=gsc[:], in_=gsc_d[b, g0 : g0 + 128, :])
                    out = outp.tile([128, Q], f16, tag="out")

                    def store(b=b, g0=g0, out=out):
                        nc.scalar.dma_start(
                            out=cost_d[b, g0 : g0 + 128, :], in_=out[:]
                        )

                    u = (mp, cls[:], gsc[:], out, store)
                    if pend is None:
                        pend = u
                    else:
                        emit_pair(pend, u)
                        pend = None
                if b == 1:
                    emit_packed()
            assert pend is None

    # enable the 2x perf path on the custom ops (byte-36 bits, pre-codegen)
    for fn in nc.m.functions:
        for bb in fn.blocks:
            for ins in bb.instructions:
                if isinstance(ins, bass_isa.InstCustomDveAnt):
                    ins.perf_max = 3

    mybir.codegen_inst_isa_subclasses(nc)  # fill ISA bytes for custom-DVE ops
    _split_multi_waits(nc)
    return nc


def _get_nc():
    if "nc" not in _cached:
        _cached["nc"] = _build_nc()
    return _cached["nc"]


def _in_maps(pred_boxes, pred_cls, gt_boxes, gt_validity):
    f16 = np.float16
    f32 = np.float32
    pb = pred_boxes.astype(f32)
    gb = gt_boxes.astype(f32)
    wp = pb[:, :, 2] - pb[:, :, 0]
    hp = pb[:, :, 3] - pb[:, :, 1]
    # clsm = -cls^T + 0.25*(Wp+Hp)  (per-column add; fp16)
    clsm = (
        -pred_cls.transpose(0, 2, 1) + (0.25 * (wp + hp))[:, None, :]
    ).astype(f16)
    pmap = np.empty((B, 5, Q), dtype=f16)
    pmap[:, 0] = -0.5 * pb[:, :, 0]
    pmap[:, 1] = -0.5 * pb[:, :, 1]
    pmap[:, 2] = -0.5 * pb[:, :, 2]
    pmap[:, 3] = -0.5 * pb[:, :, 3]
    pmap[:, 4] = 4.0 * wp * hp
    wg = gb[:, :, 2] - gb[:, :, 0]
    hg = gb[:, :, 3] - gb[:, :, 1]
    v = gt_validity.astype(f32)
    gsc = np.zeros((B, Q, 8), dtype=f32)
    gsc[:, :, 0:4] = -0.5 * gb
    gsc[:, :, 4] = 4.0 * wg * hg + 4.0 * EPS
    gsc[:, :, 5] = v
    gsc[:, :, 6] = v * (2.0 + 0.25 * (wg + hg))
    maps = []
    for c in range(N_CORES):
        sl = slice(c * B_PER, (c + 1) * B_PER)
        maps.append(
            {
                "clsm": np.ascontiguousarray(clsm[sl]),
                "pmap": np.ascontiguousarray(pmap[sl]),
                "gsc": np.ascontiguousarray(gsc[sl]),
            }
        )
    return maps


def kernel(pred_boxes, pred_cls, gt_boxes, gt_validity, _trace=False):
    from concourse import bass_utils

    nc = _get_nc()
    maps = _in_maps(pred_boxes, pred_cls, gt_boxes, gt_validity)
    res = bass_utils.run_bass_kernel_spmd(
        nc, maps, core_ids=list(range(N_CORES)), trace=_trace
    )
    out = np.concatenate(
        [res.results[c]["cost"].astype(np.float32) for c in range(N_CORES)], axis=0
    )
    if _trace:
        _cached["last_result"] = res
    return out


# revision 21
# speedup vs baseline: 1.0571x; 1.0571x over previous
"""DETR-style matcher cost matrix on 8 Trainium2 NeuronCores — v3.

cost[b, g, p] = -pred_cls[b, p, g]
                + mean(|pred_box[p] - gt_box[g]|)          (L1, 4 coords)
                + 1 - IoU + (area_c - union)/(area_c+eps)  (GIoU loss)
masked to zero where gt_validity[b, g] == 0.

Sharding: data-parallel over batch, 4 batches per core (B=32, 8 cores).
Layout per unit: [128 part = gt rows, 900 free = preds], fp16 throughout.

v5: all elementwise work is custom DVE ops authored WITH 2x perf-mode uop
programs (perf_max in byte-36 + 2x variants in the per-NEFF DVE table), so
each [128,900] op runs at ~630ns instead of ~1150ns; the no-scalar ops and
the Act reciprocals are additionally fused across unit PAIRS as [128,1800]
instructions to amortize per-instruction overhead.  Host maps are
negate-scaled (Pm = -0.5*P, Gm = -0.5*G) so every PE pass uses the same +1
identity stationary:
  wi02n = max(Pm2x2,Gm2x2) - min(Pm2x1,Gm2x1)  [W0CF]  (= -0.25*2*wi0)
  wc2n  = min(Pm2x2,Gm2x2) - max(Pm2x1,Gm2x1)  [W0F]   (= -0.25*2*wc)
  inter16 = min(wi02n,0)*min(hi02n,0)          [MINMUL] (= inter/4)
  union4e = (4Ap + (4Ag+4eps)) - 16*inter16    [SUBSCM]
  areac4  = 16*wc2n*hc2n                       [MULIMM imm=16]
  rcu4  = 1/union4e, rca4N = -1/(areac4+4eps)  [Act Reciprocal]
  iouN  = -16*inter16*rcu4 = -iou              [MULIMM imm=-16]
  t2mN  = union4e*rca4N = -(union+eps)/(areac+eps)  [MULIMM]
PE (5 passes, all stationary +1*I): psum = clsm + wi02n + hi02n + iouN +
  t2mN, where clsm = -cls^T + 0.25*(Wp+Hp) (host).  l1's per-gt part + the
  GIoU "+2" ride the out-op bias: out = V*psum + V*(2 + 0.25*(Wg+Hg)).
"""

import numpy as np

B, Q = 32, 900
N_CORES = 8
B_PER = B // N_CORES
EPS = 1e-7
_cached = {}


def _split_multi_waits(nc):
    """This neuronxcc build rejects >1 sync-wait per instruction. Split any
    instruction carrying N>1 waits by inserting N-1 wait-carrier nops before
    it on the same (in-order) engine stream."""
    import concourse.mybir as mybir

    for fn in nc.m.functions:
        for bb in fn.blocks:
            out = []
            for ins in bb.instructions:
                si = getattr(ins, "sync_info", None)
                waits = list(si.on_wait) if (si and si.on_wait) else []
                if len(waits) > 1:
                    si.on_wait = [waits[-1]]
                    for j, w in enumerate(waits[:-1]):
                        nop = mybir.InstNoOp(name=f"{ins.name}-sw{j}", ins=[], outs=[])
                        nop.engine = ins.engine
                        nop.sync_info = mybir.SyncInfo(on_wait=[w], on_update=[])
                        out.append(nop)
                out.append(ins)
            bb.instructions[:] = out


def _ensure_custom_ops():
    """Author the fused DVE ops with hand-written 2X perf-mode uop programs
    (stock-table idiom: results parked in delay lanes, WR0_LO/HI select them)
    and register them in dve_ops' tables + compile cache."""
    from concourse import dve_ops
    from concourse.dve_spec import (
        Spec, Src0, Src1, C0, C1, C2, Zero, minn, maxx, relu, lower, _has_src1,
    )
    from concourse.dve_uop import (
        DveOpSpec, UopConfig, UopDpConfig, InpSel, OutPath, OutSel, AluOp,
        AluInp, DelayInp,
    )

    A = AluInp
    D = {i: getattr(A, f"PREV_DELAY_{i}") for i in range(6)}
    PA = A.PREV_ALU_OUT

    def mk2x(lanes, blocks, lo_out, hi_out):
        u = UopConfig()
        for i, s in enumerate(lanes):
            if s is not None:
                u.enable_input(s, i)
        dps = []
        for (op, s0, s1, dmap) in blocks:
            dp = UopDpConfig()
            if op is not None:
                dp.op = op
                dp.alu_src0 = s0
                dp.alu_src1 = s1
                dp.alu_out_enable = 1
            else:
                dp.alu_out_enable = 0
            dp.delay = [DelayInp.PREV_DELAY] * 7
            dp.delay_enable = [0] * 7
            for lane, sel in dmap.items():
                dp.delay[lane] = DelayInp(sel)
                dp.delay_enable[lane] = 1
            dps.append(dp)
        assert len(dps) == 8
        u.datapath_config = dps
        u.enable_output(lo_out, OutPath.WR0_LO)
        u.enable_output(hi_out, OutPath.WR0_HI)
        return u

    # min/max-pair ops: lanes 0=x 1->d0=y 2->d1=c0 3->d2=c1 4->d3=xh 5->d4=yh
    def u2x_minmax(op_a, op_b):
        return mk2x(
            [InpSel.SRC_0, InpSel.SRC_1, InpSel.CONST_0, InpSel.CONST_1,
             InpSel.SRC_0_HI, InpSel.SRC_1_HI, None],
            [
                (op_a, PA, D[1], {0: 5, 1: 5, 2: 5, 3: 5, 4: 5}),
                (op_b, D[0], D[2], {1: 5, 2: 5, 3: 5, 4: 5, 5: 0}),
                (AluOp.SUBTRACT, D[5], PA, {1: 5, 2: 5, 3: 5, 4: 5}),
                (op_a, D[3], D[1], {2: 5, 4: 5, 5: 0}),
                (op_b, D[4], D[2], {5: 5, 0: 0}),
                (AluOp.SUBTRACT, D[0], PA, {5: 5}),
                (None, None, None, {5: 5, 0: 0}),
                (None, None, None, {5: 5, 0: 5}),
            ],
            OutSel.DELAY_5, OutSel.DELAY_0,
        )

    # MULIMM: x*y*c2 ; lanes 0=x 1->d0=y 2->d1=c2 3->d2=xh 4->d3=yh
    u2x_mul = mk2x(
        [InpSel.SRC_0, InpSel.SRC_1, InpSel.CONST_2, InpSel.SRC_0_HI,
         InpSel.SRC_1_HI, None, None],
        [
            (AluOp.MULTIPLY, PA, D[0], {1: 5, 2: 5, 3: 5}),
            (AluOp.MULTIPLY, PA, D[1], {1: 5, 2: 5, 3: 5}),
            (AluOp.MULTIPLY, D[2], D[3], {1: 5, 4: 0}),
            (AluOp.MULTIPLY, PA, D[1], {4: 5}),
            (None, None, None, {4: 5, 5: 0}),
            (None, None, None, {4: 5, 5: 5}),
            (None, None, None, {4: 5, 5: 5}),
            (None, None, None, {4: 5, 5: 5}),
        ],
        OutSel.DELAY_4, OutSel.DELAY_5,
    )

    # RELUMULF: relu(x)*relu(y) ; lanes 0=x 1->d0=y 2->d1=ZERO 3->d2=xh 4->d3=yh
    u2x_rm = mk2x(
        [InpSel.SRC_0, InpSel.SRC_1, InpSel.ZERO, InpSel.SRC_0_HI,
         InpSel.SRC_1_HI, None, None],
        [
            (AluOp.MAX, PA, D[1], {0: 5, 1: 5, 2: 5, 3: 5}),
            (AluOp.MAX, D[0], D[1], {1: 5, 2: 5, 3: 5, 5: 0}),
            (AluOp.MULTIPLY, D[5], PA, {1: 5, 2: 5, 3: 5}),
            (AluOp.MAX, D[2], D[1], {1: 5, 3: 5, 4: 0}),
            (AluOp.MAX, D[3], D[1], {4: 5, 0: 0}),
            (AluOp.MULTIPLY, D[0], PA, {4: 5}),
            (None, None, None, {4: 5, 5: 0}),
            (None, None, None, {4: 5, 5: 5}),
        ],
        OutSel.DELAY_4, OutSel.DELAY_5,
    )

    # SUBSC: (x + c0) - y ; lanes 0=x 1->d0=y 2->d1=c0 3->d2=xh 4->d3=yh
    u2x_sub = mk2x(
        [InpSel.SRC_0, InpSel.SRC_1, InpSel.CONST_0, InpSel.SRC_0_HI,
         InpSel.SRC_1_HI, None, None],
        [
            (AluOp.ADD, PA, D[1], {0: 5, 1: 5, 2: 5, 3: 5}),
            (AluOp.SUBTRACT, PA, D[0], {1: 5, 2: 5, 3: 5}),
            (AluOp.ADD, D[2], D[1], {3: 5, 4: 0}),
            (AluOp.SUBTRACT, PA, D[3], {4: 5}),
            (None, None, None, {4: 5, 5: 0}),
            (None, None, None, {4: 5, 5: 5}),
            (None, None, None, {4: 5, 5: 5}),
            (None, None, None, {4: 5, 5: 5}),
        ],
        OutSel.DELAY_4, OutSel.DELAY_5,
    )

    # MINMUL: min(x,0)*min(y,0) ; lanes 0=x 1->d0=y 2->d1=ZERO 3->d2=xh 4->d3=yh
    u2x_mm = mk2x(
        [InpSel.SRC_0, InpSel.SRC_1, InpSel.ZERO, InpSel.SRC_0_HI,
         InpSel.SRC_1_HI, None, None],
        [
            (AluOp.MIN, PA, D[1], {0: 5, 1: 5, 2: 5, 3: 5}),
            (AluOp.MIN, D[0], D[1], {1: 5, 2: 5, 3: 5, 5: 0}),
            (AluOp.MULTIPLY, D[5], PA, {1: 5, 2: 5, 3: 5}),
            (AluOp.MIN, D[2], D[1], {1: 5, 3: 5, 4: 0}),
            (AluOp.MIN, D[3], D[1], {4: 5, 0: 0}),
            (AluOp.MULTIPLY, D[0], PA, {4: 5}),
            (None, None, None, {4: 5, 5: 0}),
            (None, None, None, {4: 5, 5: 5}),
        ],
        OutSel.DELAY_4, OutSel.DELAY_5,
    )

    # SUBSCM: (x + c0) - y*c2
    # lanes: 0=y 1->d0=x 2->d1=c0 3->d2=c2 4->d3=yh 5->d4=xh
    u2x_subm = mk2x(
        [InpSel.SRC_1, InpSel.SRC_0, InpSel.CONST_0, InpSel.CONST_2,
         InpSel.SRC_1_HI, InpSel.SRC_0_HI, None],
        [
            (AluOp.MULTIPLY, PA, D[2], {0: 5, 1: 5, 2: 5, 3: 5, 4: 5}),
            (AluOp.ADD, D[0], D[1], {1: 5, 2: 5, 3: 5, 4: 5, 5: 0}),
            (AluOp.SUBTRACT, PA, D[5], {1: 5, 2: 5, 3: 5, 4: 5}),
            (AluOp.MULTIPLY, D[3], D[2], {1: 5, 4: 5, 5: 0}),
            (AluOp.ADD, D[4], D[1], {5: 5, 0: 0}),
            (AluOp.SUBTRACT, PA, D[0], {5: 5}),
            (None, None, None, {5: 5, 0: 0}),
            (None, None, None, {5: 5, 0: 5}),
        ],
        OutSel.DELAY_5, OutSel.DELAY_0,
    )

    def author(name, body, ref, u2):
        if name in dve_ops._SUB_OPCODE_FOR_NAME:
            return
        spec = Spec(body=body, reference=ref)
        row = max(dve_ops._SUB_OPCODE_FOR_NAME.values()) + 1
        shas = {}
        for ver in ("v3", "v4"):
            uops = lower(spec, ver=ver)
            assert len(uops) == 1, (name, len(uops))
            import copy as _copy

            u2v = _copy.deepcopy(u2)
            u2v.trigger = uops[0].trigger
            u2v.next_uop = uops[0].next_uop
            u2v.repeat_count = uops[0].repeat_count
            u2v.require_inp0 = uops[0].require_inp0
            u2v.require_inp1 = uops[0].require_inp1
            s = DveOpSpec(
                name=name, opcode=row, uops=uops,
                uops_2x=[u2v], uops_2x_2p=[u2v], uops_4x=[u2v],
                perf_max=3, rd1_en=_has_src1(spec),
            )
            shas[ver] = s.sha(ver)
            dve_ops._COMPILE_CACHE[(name, ver)] = s
        op = dve_ops.DveOp(name, spec, False, shas)
        dve_ops.OPS.append(op)
        dve_ops.CUSTOM_DVE_SPECS[name] = spec
        dve_ops._SUB_OPCODE_FOR_NAME[name] = row

    author(
        "W0F_ANT",
        minn(Src0, C0) - maxx(Src1, C1),
        lambda in0, in1, s0, s1, imm2: np.minimum(in0, s0) - np.maximum(in1, s1),
        u2x_minmax(AluOp.MIN, AluOp.MAX),
    )
    author(
        "W0CF_ANT",
        maxx(Src0, C0) - minn(Src1, C1),
        lambda in0, in1, s0, s1, imm2: np.maximum(in0, s0) - np.minimum(in1, s1),
        u2x_minmax(AluOp.MAX, AluOp.MIN),
    )
    author(
        "RELUMULF_ANT",
        relu(Src0) * relu(Src1),
        lambda in0, in1, s0, s1, imm2: np.maximum(in0, 0.0) * np.maximum(in1, 0.0),
        u2x_rm,
    )
    author(
        "SUBSC_ANT",
        (Src0 + C0) - Src1,
        lambda in0, in1, s0, s1, imm2: (in0 + s0) - in1,
        u2x_sub,
    )
    author(
        "MINMUL_ANT",
        minn(Src0, Zero) * minn(Src1, Zero),
        lambda in0, in1, s0, s1, imm2: np.minimum(in0, 0.0) * np.minimum(in1, 0.0),
        u2x_mm,
    )
    author(
        "SUBSCM_ANT",
        (Src0 + C0) - Src1 * C2,
        lambda in0, in1, s0, s1, imm2: (in0 + s0) - in1 * imm2,
        u2x_subm,
    )
    author(
        "MULIMM_ANT",
        Src0 * Src1 * C2,
        lambda in0, in1, s0, s1, imm2: in0 * in1 * imm2,
        u2x_mul,
    )


def _by_name(dve_ops, name):
    for op in dve_ops.OPS:
        if op.name == name:
            return op
    raise KeyError(name)


def _raw_act(nc, out, in_, func, bias=0.0, scale=1.0):
    """activation() without the Reciprocal accuracy ban (measured ~1e-5)."""
    import concourse.bass as bass
    from concourse import mybir

    inputs = [nc.scalar.lower_ap(in_)]
    for arg in (bias, scale, 0.0):
        if isinstance(arg, bass.AP):
            inputs.append(nc.scalar.lower_ap(arg))
        else:
            inputs.append(mybir.ImmediateValue(dtype=mybir.dt.float32, value=arg))
    return nc.scalar.add_instruction(
        mybir.InstActivation(
            name=nc.get_next_instruction_name(),
            func=func,
            ins=inputs,
            outs=[nc.scalar.lower_ap(out)],
        )
    )


def _build_nc():
    import concourse.bass as bass
    from concourse import mybir, dve_ops, bass_isa
    from concourse.tile import TileContext
    from concourse.masks import make_identity

    _ensure_custom_ops()
    W0F = _by_name(dve_ops, "W0F_ANT")
    W0CF = _by_name(dve_ops, "W0CF_ANT")
    RM = _by_name(dve_ops, "MINMUL_ANT")
    SUB = _by_name(dve_ops, "SUBSCM_ANT")
    MUL = _by_name(dve_ops, "MULIMM_ANT")

    f32 = mybir.dt.float32
    f16 = mybir.dt.float16
    Act = mybir.ActivationFunctionType

    nc = bass.Bass()
    # host-packed inputs
    cls_d = nc.dram_tensor("clsm", [B_PER, Q, Q], f16, kind="ExternalInput")
    pmap_d = nc.dram_tensor("pmap", [B_PER, 5, Q], f16, kind="ExternalInput")
    gsc_d = nc.dram_tensor("gsc", [B_PER, Q, 8], f32, kind="ExternalInput")
    cost_d = nc.dram_tensor("cost", [B_PER, Q, Q], f16, kind="ExternalOutput")

    with TileContext(nc) as tc:
        with (
            tc.tile_pool(name="const", bufs=1) as constp,
            tc.tile_pool(name="batch", bufs=3) as batchp,
            tc.tile_pool(name="unit", bufs=3) as up,
            tc.tile_pool(name="cls", bufs=6) as clsp,
            tc.tile_pool(name="outp", bufs=4) as outp,
            tc.tile_pool(name="ps", bufs=4, space="PSUM") as psp,
        ):
            # stationary matrices for PE accumulate passes
            ident = constp.tile([128, 128], f32)
            make_identity(nc, ident)
            stat_p1 = constp.tile([128, 128], f16)
            nc.vector.tensor_scalar_mul(stat_p1[:], ident[:], 1.0)

            def emit_scalar_stage(mp, gsc, wi02, hi02, wc2, hc2):
                """Per-unit scalar-carrying customs into (slices of) tiles."""
                G2x1 = gsc[:, 0:1]
                G2y1 = gsc[:, 1:2]
                G2x2 = gsc[:, 2:3]
                G2y2 = gsc[:, 3:4]
                cd = nc.vector._custom_dve
                cd(W0CF, out=wi02, in0=mp["P2x2"], in1=mp["P2x1"],
                   s0=G2x2, s1=G2x1)
                cd(W0CF, out=hi02, in0=mp["P2y2"], in1=mp["P2y1"],
                   s0=G2y2, s1=G2y1)
                cd(W0F, out=wc2, in0=mp["P2x2"], in1=mp["P2x1"],
                   s0=G2x2, s1=G2x1)
                cd(W0F, out=hc2, in0=mp["P2y2"], in1=mp["P2y1"],
                   s0=G2y2, s1=G2y1)

            def emit_pe_out(cls, gsc, out, wi02, hi02, iouN, t2mN):
                """PE accumulation + V-scale out for one unit; inputs are
                [128,900] APs (possibly slices of fused double tiles)."""
                V = gsc[:, 5:6]
                bV = gsc[:, 6:7]
                psA = psp.tile([128, 512], f32, tag="psA")
                psB = psp.tile([128, Q - 512], f32, tag="psB")
                passes = [
                    (stat_p1, cls, True),
                    (stat_p1, wi02, False),
                    (stat_p1, hi02, False),
                    (stat_p1, iouN, False),
                    (stat_p1, t2mN, False),
                ]
                for k, (st, x, first) in enumerate(passes):
                    last = k == len(passes) - 1
                    nc.tensor.matmul(
                        psA[:], st[:], x[:, 0:512], start=first, stop=last,
                        skip_group_check=True,
                    )
                    nc.tensor.matmul(
                        psB[:], st[:], x[:, 512:Q], start=first, stop=last,
                        skip_group_check=True,
                    )
                nc.scalar.activation(
                    out[:, 0:512], psA[:], Act.Identity, scale=V, bias=bV
                )
                nc.scalar.activation(
                    out[:, 512:Q], psB[:], Act.Identity, scale=V, bias=bV
                )

            def emit_pair(ua, ub):
                """Two units fused: the no-scalar customs + Act reciprocals
                run once over [128,1800] double-width tiles."""
                cd = nc.vector._custom_dve
                Q2 = 2 * Q
                wi02 = up.tile([128, Q2], f16, tag="wi02")
                hi02 = up.tile([128, Q2], f16, tag="hi02")
                wc2 = up.tile([128, Q2], f16, tag="wc2")
                hc2 = up.tile([128, Q2], f16, tag="hc2")
                for (mp, cls, gsc, out, store), o in ((ua, 0), (ub, Q)):
                    emit_scalar_stage(
                        mp, gsc, wi02[:, o : o + Q], hi02[:, o : o + Q],
                        wc2[:, o : o + Q], hc2[:, o : o + Q],
                    )
                inter4 = up.tile([128, Q2], f16, tag="inter4")
                cd(RM, out=inter4[:], in0=wi02[:], in1=hi02[:])
                areac4 = up.tile([128, Q2], f16, tag="areac4")
                cd(MUL, out=areac4[:], in0=wc2[:], in1=hc2[:], imm2=16.0)
                union4e = up.tile([128, Q2], f16, tag="union4e")
                for (mp, cls, gsc, out, store), o in ((ua, 0), (ub, Q)):
                    cd(SUB, out=union4e[:, o : o + Q], in0=mp["Ap4"],
                       in1=inter4[:, o : o + Q], s0=gsc[:, 4:5], imm2=16.0)
                rcu4 = up.tile([128, Q2], f16, tag="rcu4")
                _raw_act(nc, rcu4[:], union4e[:], Act.Reciprocal)
                rca4N = up.tile([128, Q2], f16, tag="rca4N")
                _raw_act(nc, rca4N[:], areac4[:], Act.Reciprocal,
                         bias=-4 * EPS, scale=-1.0)
                iouN = up.tile([128, Q2], f16, tag="iouN")
                cd(MUL, out=iouN[:], in0=inter4[:], in1=rcu4[:], imm2=-16.0)
                t2mN = up.tile([128, Q2], f16, tag="t2mN")
                cd(MUL, out=t2mN[:], in0=union4e[:], in1=rca4N[:], imm2=1.0)
                for (mp, cls, gsc, out, store), o in ((ua, 0), (ub, Q)):
                    emit_pe_out(
                        cls, gsc, out, wi02[:, o : o + Q], hi02[:, o : o + Q],
                        iouN[:, o : o + Q], t2mN[:, o : o + Q],
                    )
                    store()

            def emit_unit(mp, cls, gsc, out, tag):
                """Unfused single unit (packed remainder)."""
                cd = nc.vector._custom_dve
                wi02 = constp.tile([128, Q], f16, tag="swi02")
                hi02 = constp.tile([128, Q], f16, tag="shi02")
                wc2 = constp.tile([128, Q], f16, tag="swc2")
                hc2 = constp.tile([128, Q], f16, tag="shc2")
                emit_scalar_stage(mp, gsc, wi02[:], hi02[:], wc2[:], hc2[:])
                inter4 = constp.tile([128, Q], f16, tag="sinter4")
                cd(RM, out=inter4[:], in0=wi02[:], in1=hi02[:])
                union4e = constp.tile([128, Q], f16, tag="sunion4e")
                cd(SUB, out=union4e[:], in0=mp["Ap4"], in1=inter4[:],
                   s0=gsc[:, 4:5], imm2=16.0)
                areac4 = constp.tile([128, Q], f16, tag="sareac4")
                cd(MUL, out=areac4[:], in0=wc2[:], in1=hc2[:], imm2=16.0)
                rcu4 = constp.tile([128, Q], f16, tag="srcu4")
                _raw_act(nc, rcu4[:], union4e[:], Act.Reciprocal)
                rca4N = constp.tile([128, Q], f16, tag="srca4N")
                _raw_act(nc, rca4N[:], areac4[:], Act.Reciprocal,
                         bias=-4 * EPS, scale=-1.0)
                iouN = constp.tile([128, Q], f16, tag="siouN")
                cd(MUL, out=iouN[:], in0=inter4[:], in1=rcu4[:], imm2=-16.0)
                t2mN = constp.tile([128, Q], f16, tag="st2mN")
                cd(MUL, out=t2mN[:], in0=union4e[:], in1=rca4N[:], imm2=1.0)
                emit_pe_out(cls, gsc, out, wi02[:], hi02[:], iouN[:], t2mN[:])

            # ---------------- packed remainder unit ----------------
            # partitions 4b..4b+3 = batch b rows 896:900; rest = filler
            def emit_packed():
                mpR = {}
                for k, nm in enumerate(("P2x1", "P2y1", "P2x2", "P2y2", "Ap4")):
                    t = constp.tile([128, Q], f16, tag=f"mapR{nm}", name=f"mapR{nm}")
                    for b in range(B_PER):
                        src = pmap_d[b, k][:]
                        rows = 4 if b < 3 else 116
                        bcast = bass.AP(
                            tensor=src.tensor, offset=src.offset,
                            ap=[[0, rows]] + list(src.ap),
                        )
                        nc.sync.dma_start(out=t[4 * b : 4 * b + rows, :], in_=bcast)
                    mpR[nm] = t[:]

                gscR = constp.tile([128, 8], f32, tag="gscR")
                # filler gt box: G2x1=0.2, G2x2=0.3 etc keeps union/areac
                # positive so the Reciprocals stay finite on filler rows
                nc.gpsimd.memset(gscR[:], 0.2)
                nc.gpsimd.memset(gscR[:, 2:4], 0.3)
                for b in range(B_PER):
                    nc.sync.dma_start(
                        out=gscR[4 * b : 4 * b + 4, :], in_=gsc_d[b, 896:900, :]
                    )
                clsR = constp.tile([128, Q], f16, tag="clsR")
                nc.gpsimd.memset(clsR[:], 0.0)
                for b in range(B_PER):
                    nc.sync.dma_start(
                        out=clsR[4 * b : 4 * b + 4, :], in_=cls_d[b, 896:900, :]
                    )
                outR = outp.tile([128, Q], f16, tag="outR")
                emit_unit(mpR, clsR[:], gscR[:], outR, "packed")
                for b in range(B_PER):
                    nc.scalar.dma_start(
                        out=cost_d[b, 896:900, :], in_=outR[4 * b : 4 * b + 4, :]
                    )

            # ---------------- full units, emitted in fused pairs --------
            pend = None
            for b in range(B_PER):
                mp = {}
                for k, nm in enumerate(("P2x1", "P2y1", "P2x2", "P2y2", "Ap4")):
                    t = batchp.tile([128, Q], f16, tag=f"map{nm}", name=f"map{nm}")
                    src = pmap_d[b, k][:]
                    bcast = bass.AP(
                        tensor=src.tensor, offset=src.offset,
                        ap=[[0, 128]] + list(src.ap),
                    )
                    nc.sync.dma_start(out=t[:], in_=bcast)
                    mp[nm] = t[:]

                for t in range(7):
                    g0 = t * 128
                    cls = clsp.tile([128, Q], f16, tag="cls")
                    nc.sync.dma_start(out=cls[:], in_=cls_d[b, g0 : g0 + 128, :])
                    gsc = clsp.tile([128, 8], f32, tag="gsc")
                    nc.sync.dma_start(out=gsc[:], in_=gsc_d[b, g0 : g0 + 128, :])
                    out = outp.tile([128, Q], f16, tag="out")

                    def store(b=b, g0=g0, out=out):
                        nc.scalar.dma_start(
                            out=cost_d[b, g0 : g0 + 128, :], in_=out[:]
                        )

                    u = (mp, cls[:], gsc[:], out, store)
                    if pend is None:
                        pend = u
                    else:
                        emit_pair(pend, u)
                        pend = None
                if b == 1:
                    emit_packed()
            assert pend is None

    # enable the 2x perf path on the custom ops (byte-36 bits, pre-codegen)
    for fn in nc.m.functions:
        for bb in fn.blocks:
            for ins in bb.instructions:
                if isinstance(ins, bass_isa.InstCustomDveAnt):
                    ins.perf_max = 3

    mybir.codegen_inst_isa_subclasses(nc)  # fill ISA bytes for custom-DVE ops
    _split_multi_waits(nc)
    return nc


def _get_nc():
    if "nc" not in _cached:
        _cached["nc"] = _build_nc()
    return _cached["nc"]


def _in_maps(pred_boxes, pred_cls, gt_boxes, gt_validity):
    f16 = np.float16
    f32 = np.float32
    pb = pred_boxes.astype(f32)
    gb = gt_boxes.astype(f32)
    wp = pb[:, :, 2] - pb[:, :, 0]
    hp = pb[:, :, 3] - pb[:, :, 1]
    # clsm = -cls^T + 0.25*(Wp+Hp)  (per-column add; fp16)
    clsm = (
        -pred_cls.transpose(0, 2, 1) + (0.25 * (wp + hp))[:, None, :]
    ).astype(f16)
    pmap = np.empty((B, 5, Q), dtype=f16)
    pmap[:, 0] = -0.5 * pb[:, :, 0]
    pmap[:, 1] = -0.5 * pb[:, :, 1]
    pmap[:, 2] = -0.5 * pb[:, :, 2]
    pmap[:, 3] = -0.5 * pb[:, :, 3]
    pmap[:, 4] = 4.0 * wp * hp
    wg = gb[:, :, 2] - gb[:, :, 0]
    hg = gb[:, :, 3] - gb[:, :, 1]
    v = gt_validity.astype(f32)
    gsc = np.zeros((B, Q, 8), dtype=f32)
    gsc[:, :, 0:4] = -0.5 * gb
    gsc[:, :, 4] = 4.0 * wg * hg + 4.0 * EPS
    gsc[:, :, 5] = v
    gsc[:, :, 6] = v * (2.0 + 0.25 * (wg + hg))
    maps = []
    for c in range(N_CORES):
        sl = slice(c * B_PER, (c + 1) * B_PER)
        maps.append(
            {
                "clsm": np.ascontiguousarray(clsm[sl]),
                "pmap": np.ascontiguousarray(pmap[sl]),
                "gsc": np.ascontiguousarray(gsc[sl]),
            }
        )
    return maps


def kernel(pred_boxes, pred_cls, gt_boxes, gt_validity, _trace=False):
    from concourse import bass_utils

    nc = _get_nc()
    maps = _in_maps(pred_boxes, pred_cls, gt_boxes, gt_validity)
    res = bass_utils.run_bass_kernel_spmd(
        nc, maps, core_ids=list(range(N_CORES)), trace=_trace
    )
    out = np.concatenate(
        [res.results[c]["cost"].astype(np.float32) for c in range(N_CORES)], axis=0
    )
    if _trace:
        _cached["last_result"] = res
    return out
